# revision 16
# baseline (speedup 1.0000x reference)
"""Causal self-attention (B=2, T=2048, C=1024, H=16, RoPE) on 8 TRN2 cores.

Sharding: data-parallel over B (2 groups of 4 cores) x tensor-parallel over
heads (4 heads per core). Each core computes q/k/v projections for its heads,
RoPE, causal attention, and its partial output projection; the host sums the
4 partial projections per batch and adds bp.

Schedule: projections for T-block tb+1 and the output projection for block
qb-1 are emitted interleaved with the attention chunk loop of block qb, so
TensorE matmul work fills the gaps while ScalarE works through the softmax
exps (the Tile scheduler pops ready work by emission priority). Input DMAs
are batched into a handful of multi-part descriptors issued on the idle
Sync engine so compute starts ~4us in instead of after a bulk load.

Layout choices (per core):
  - x resident in SBUF as one [128, 8*2048] tile (contraction chunks side
    by side); q, k produced TRANSPOSED: qT/kT [256=4heads*64, T] via
    lhsT=W, rhs=xT. Head-dim pairs are pre-permuted (evens|odds) in the
    weights so RoPE needs no strided access; the pair-swap is a constant
    permutation matmul (J), combined on VectorE in bf16.
  - v produced NON-transposed [T, 4 heads]; per head pair the SBUF layout
    is [v_even(64) | 1 | 1 | 0*63 | v_odd(64)] so BOTH heads' PV matmuls
    also emit their softmax denominator rows (even: psum row 64, odd: psum
    row 0) with no extra reduction work.
  - scores computed transposed: ST[tk, tq] = k_rot @ q_rot^T per head
    (two heads row-packed into PE quadrants), softmax-exp elementwise on
    ScalarE (scale folded), causal handling = per-chunk left-trim of the
    matmul/exp range + a fixed 128x128 triangle multiply on GpSimd.
  - denominators: staged bf16, broadcast over partitions by a constant
    matmul (EA), reciprocal via the fast approximate DVE op.
"""

import math

import numpy as np
import ml_dtypes

import concourse.bass as bass
import concourse.bacc as bacc
import concourse.mybir as mybir
from concourse.tile import TileContext
from concourse.bass_utils import run_bass_kernel_spmd

BF16 = mybir.dt.bfloat16
F32 = mybir.dt.float32
NPBF16 = ml_dtypes.bfloat16

N_CORES = 8
P = 128

_UNIFIED_ACT_SET = "natural_log_exp_and_others"


def _patch_act_tables():
    import concourse.hw_specs as _hw
    import concourse.bacc as _bacc
    if getattr(_bacc, "_act_tables_patched", False):
        return
    _orig = _hw.get_activation_tables

    def _gat(arch):
        tabs = _orig(arch)
        if _UNIFIED_ACT_SET in tabs:
            keep = tabs[_UNIFIED_ACT_SET]
            drop = {
                mybir.ActivationFunctionType.Exp,
                mybir.ActivationFunctionType.Copy,
            } & keep
            for name, fns in tabs.items():
                if name != _UNIFIED_ACT_SET:
                    for f in drop:
                        fns.discard(f)
        return tabs

    _bacc.get_activation_tables = _gat
    _bacc._act_tables_patched = True


def build_attention_kernel(nc, T=2048, C=1024, n_heads=4, hd=64):
    """Emit the per-core kernel. Returns nothing; tensors are declared on nc."""
    _patch_act_tables()
    HD = n_heads * hd            # 256: local head dims
    KC = C // P                  # 8: contraction chunks for projections
    NJC = HD // P                # 2: partition tiles of qT/kT (head pairs)
    TQB = 512                    # tq block for scores/PV
    NQB = T // TQB               # 4
    NTT = T // P                 # 16: t tiles for v
    VW = 2 * hd + 65             # 193: per-pair v columns
    scale = 1.0 / math.sqrt(hd)

    # ---- DRAM I/O (host pre-swizzled so every DMA is 2D-contiguous) ----
    # xq[tb*128+p, k*512+c] = x[tb*512+c, k*128+p]
    xq = nc.declare_dram_parameter("xq", [NQB * P, KC * TQB], BF16,
                                   isOutput=False)
    # w*[p, k*256+c] = W.T[k*128+p, c] (rows RoPE-permuted for q/k)
    wqT = nc.declare_dram_parameter("wqT", [P, KC * HD], BF16, isOutput=False)
    wkT = nc.declare_dram_parameter("wkT", [P, KC * HD], BF16, isOutput=False)
    wvT = nc.declare_dram_parameter("wvT", [P, KC * HD], BF16, isOutput=False)
    wpT = nc.declare_dram_parameter("wpT", [P, NJC * C], BF16, isOutput=False)
    cosq = nc.declare_dram_parameter("cosq", [P, T], BF16, isOutput=False)
    sinsq = nc.declare_dram_parameter("sinsq", [P, T], BF16, isOutput=False)
    # cmat = [jmat | tri | ea]
    cmat = nc.declare_dram_parameter("cmat", [P, 3 * P], BF16, isOutput=False)
    # bias = [bqT(2) | bkT(2) | bvb(256)]
    bias = nc.declare_dram_parameter("bias", [P, 2 * NJC + HD], F32,
                                     isOutput=False)
    z = nc.declare_dram_parameter("z", [T, C], BF16, isOutput=True)

    with TileContext(nc) as tc:
        import contextlib

        with contextlib.ExitStack() as ctx:
            # ---- persistent SBUF pools ----
            pc = ctx.enter_context(tc.tile_pool(name="const", bufs=1))
            px = ctx.enter_context(tc.tile_pool(name="x", bufs=1))
            pw = ctx.enter_context(tc.tile_pool(name="w", bufs=1))
            pqk = ctx.enter_context(tc.tile_pool(name="qk", bufs=1))
            pv = ctx.enter_context(tc.tile_pool(name="v", bufs=1))
            py = ctx.enter_context(tc.tile_pool(name="y", bufs=1))
            # transient SBUF pools
            praw = ctx.enter_context(tc.tile_pool(name="raw", bufs=4))
            pjq = ctx.enter_context(tc.tile_pool(name="jq", bufs=4))
            prt = ctx.enter_context(tc.tile_pool(name="ropetmp", bufs=6))
            pexp = ctx.enter_context(tc.tile_pool(name="exp", bufs=8))
            prcp = ctx.enter_context(tc.tile_pool(name="rcp", bufs=2))
            pzev = ctx.enter_context(tc.tile_pool(name="zev", bufs=3))
            # PSUM pools: 2*2 + 2*1 + 2*1 = 8 banks
            pmm = ctx.enter_context(
                tc.tile_pool(name="mm", bufs=2, space="PSUM"))
            pyt = ctx.enter_context(
                tc.tile_pool(name="yt", bufs=1, space="PSUM"))
            pps = ctx.enter_context(
                tc.tile_pool(name="pp", bufs=2, space="PSUM"))

            # ---- input DMAs: contiguous 2D transfers spread over engines
            # so issue-time and queue draining parallelize; x part 0 and
            # wv go first (v-proj needs them)
            t_x = px.tile([P, NQB * KC * TQB], BF16, tag="x")
            for tb in range(NQB):
                nc.sync.dma_start(
                    t_x[:, tb * KC * TQB:(tb + 1) * KC * TQB],
                    xq[tb * P:(tb + 1) * P, :])
            t_wv = pw.tile([P, KC * HD], BF16, tag="wv")
            nc.gpsimd.dma_start(t_wv[:], wvT[:])
            t_wq = pw.tile([P, KC * HD], BF16, tag="wq")
            nc.gpsimd.dma_start(t_wq[:], wqT[:])
            t_wk = pw.tile([P, KC * HD], BF16, tag="wk")
            nc.gpsimd.dma_start(t_wk[:], wkT[:])
            t_cos = pc.tile([P, T], BF16, tag="cos")
            nc.gpsimd.dma_start(t_cos[:], cosq[:])
            t_sin = pc.tile([P, T], BF16, tag="sin")
            nc.gpsimd.dma_start(t_sin[:], sinsq[:])
            t_cm = pc.tile([P, 3 * P], BF16, tag="cm")
            nc.scalar.dma_start(t_cm[:], cmat[:])
            t_j = t_cm[:, 0:P]
            t_tri = t_cm[:, P:2 * P]
            t_ea = t_cm[:, 2 * P:3 * P]
            t_bias = pc.tile([P, 2 * NJC + HD], F32, tag="bias")
            nc.scalar.dma_start(t_bias[:], bias[:])
            t_bq = t_bias[:, 0:NJC]
            t_bk = t_bias[:, NJC:2 * NJC]
            t_bv = t_bias[:, 2 * NJC:2 * NJC + HD]
            t_wp = pw.tile([P, NJC * C], BF16, tag="wp")
            nc.scalar.dma_start(t_wp[:], wpT[:])

            # denominator staging tiles (rows 0/64 carry data, rest stay 1.0)
            t_scp = []
            for i in range(2):
                s = pc.tile([P, TQB], BF16, tag=f"scp{i}", name=f"scp{i}")
                nc.vector.memset(s[:], 1.0)
                t_scp.append(s)

            # persistent targets
            t_qrot = [pqk.tile([P, T], BF16, tag=f"qr{jc}", name=f"qrot{jc}")
                      for jc in range(NJC)]
            t_krot = [pqk.tile([P, T], BF16, tag=f"kr{jc}", name=f"krot{jc}")
                      for jc in range(NJC)]
            t_yn = [py.tile([P, T], BF16, tag=f"yn{jc}", name=f"yn{jc}")
                    for jc in range(NJC)]
            t_v = [pv.tile([P, NJC * VW], BF16, tag=f"v{tt}", name=f"v{tt}")
                   for tt in range(NTT)]

            def wsl(w, k, jc):
                return w[:, k * HD + jc * P:k * HD + (jc + 1) * P]

            def xsl(k, lo, hi):
                # x SBUF layout: [p, tb*4096 + k*512 + (t - tb*512)];
                # [lo, hi) must lie within one tb block
                tb = lo // TQB
                base = tb * KC * TQB + k * TQB - tb * TQB
                return t_x[:, base + lo:base + hi]

            # ---- emission quanta ----
            def q_vproj(tt):
                def go():
                    vps = pps.tile([P, TQB], F32, tag="pp")
                    for k in range(KC):
                        nc.tensor.matmul(
                            vps[:, 0:HD],
                            lhsT=xsl(k, tt * P, (tt + 1) * P),
                            rhs=t_wv[:, k * HD:(k + 1) * HD],
                            start=(k == 0),
                            stop=(k == KC - 1),
                        )
                    vt = t_v[tt]
                    # heads p*2 (even) at cols [p*VW, p*VW+64)
                    dst_e = bass.AP(
                        vt.tensor, vt[:].offset,
                        [vt[:].ap[0], [VW, NJC], [1, hd]])
                    src_e = bass.AP(
                        vps.tensor, vps[:].offset,
                        [vps[:].ap[0], [2 * hd, NJC], [1, hd]])
                    b_e = bass.AP(
                        t_bv.tensor, t_bv.offset,
                        [t_bv.ap[0], [2 * hd, NJC], [1, hd]])
                    nc.vector.tensor_add(dst_e, src_e, b_e)
                    # heads p*2+1 (odd) at cols [p*VW+129, p*VW+193)
                    dst_o = bass.AP(
                        vt.tensor, vt[:].offset + (2 * hd + 1),
                        [vt[:].ap[0], [VW, NJC], [1, hd]])
                    src_o = bass.AP(
                        vps.tensor, vps[:].offset + hd,
                        [vps[:].ap[0], [2 * hd, NJC], [1, hd]])
                    b_o = bass.AP(
                        t_bv.tensor, t_bv.offset + hd,
                        [t_bv.ap[0], [2 * hd, NJC], [1, hd]])
                    nc.vector.tensor_add(dst_o, src_o, b_o)
                    # ones at cols {64, 65}, zeros at [66, 129) per pair
                    ones_ap = bass.AP(
                        vt.tensor, vt[:].offset + hd,
                        [vt[:].ap[0], [VW, NJC], [1, 2]])
                    nc.gpsimd.memset(ones_ap, 1.0)
                    zer_ap = bass.AP(
                        vt.tensor, vt[:].offset + hd + 2,
                        [vt[:].ap[0], [VW, NJC], [1, hd - 1]])
                    nc.gpsimd.memset(zer_ap, 0.0)
                return go

            def q_qkproj(dst, wten, bias, jc, tb):
                def go():
                    lo, hi = tb * TQB, (tb + 1) * TQB
                    qps = pps.tile([P, TQB], F32, tag="pp")
                    for k in range(KC):
                        nc.tensor.matmul(
                            qps[:],
                            lhsT=wsl(wten, k, jc),
                            rhs=xsl(k, lo, hi),
                            start=(k == 0),
                            stop=(k == KC - 1),
                        )
                    raw = praw.tile([P, TQB], BF16, tag="raw")
                    nc.vector.tensor_scalar_add(
                        raw[:], qps[:], bias[:, jc:jc + 1])
                    jps = pps.tile([P, TQB], F32, tag="pp")
                    nc.tensor.matmul(jps[:], lhsT=t_j, rhs=raw[:])
                    jq = pjq.tile([P, TQB], BF16, tag="jq")
                    with nc.allow_low_precision(reason="bf16 rope"):
                        nc.any.tensor_copy(jq[:], jps[:])
                        tc1 = prt.tile([P, TQB], BF16, tag="rt")
                        nc.vector.tensor_mul(tc1[:], raw[:], t_cos[:, lo:hi])
                        tc2 = prt.tile([P, TQB], BF16, tag="rt")
                        nc.vector.tensor_mul(tc2[:], jq[:], t_sin[:, lo:hi])
                        nc.vector.tensor_add(dst[:, lo:hi], tc1[:], tc2[:])
                return go

            def q_chunk(qb, hp, kc, n_kc, yt_a, yt_b):
                def go():
                    s0 = max(0, kc * P - qb * TQB)
                    sc = pmm.tile([P, 2 * TQB], F32, tag="sc")
                    for hl in range(2):
                        nc.tensor.matmul(
                            sc[:, hl * TQB + s0:(hl + 1) * TQB],
                            lhsT=t_krot[hp][
                                hl * hd:(hl + 1) * hd,
                                kc * P:(kc + 1) * P],
                            rhs=t_qrot[hp][
                                hl * hd:(hl + 1) * hd,
                                qb * TQB + s0:(qb + 1) * TQB],
                        )
                    ex = pexp.tile([P, 2 * TQB], BF16, tag="exp")
                    sc3 = sc[:].rearrange("p (h w) -> p h w", h=2)
                    ex3 = ex[:].rearrange("p (h w) -> p h w", h=2)
                    nc.scalar.activation(
                        ex3[:, :, s0:TQB],
                        sc3[:, :, s0:TQB],
                        mybir.ActivationFunctionType.Exp,
                        scale=scale,
                    )
                    # diagonal 128-wide triangle mask (tk<=tq kept)
                    if kc * P >= qb * TQB:
                        tri3 = bass.AP(
                            t_tri.tensor, t_tri.offset,
                            [t_tri.ap[0], [0, 2], t_tri.ap[1]],
                        )
                        nc.gpsimd.tensor_mul(
                            ex3[:, :, s0:s0 + P],
                            ex3[:, :, s0:s0 + P],
                            tri3,
                        )
                    # P @ V; both heads' denominators ride along:
                    # even head -> yt_a row 64, odd head -> yt_b row 0
                    vt = t_v[kc]
                    nc.tensor.matmul(
                        yt_a[0:hd + 1, s0:TQB],
                        lhsT=vt[:, hp * VW:hp * VW + hd + 1],
                        rhs=ex[:, s0:TQB],
                        start=(kc == 0),
                        stop=(kc == n_kc - 1),
                        skip_group_check=True,
                    )
                    nc.tensor.matmul(
                        yt_b[:, s0:TQB],
                        lhsT=vt[:, hp * VW + hd + 1:(hp + 1) * VW],
                        rhs=ex[:, TQB + s0:2 * TQB],
                        start=(kc == 0),
                        stop=(kc == n_kc - 1),
                        skip_group_check=True,
                    )
                return go

            def q_tail(qb, hp, yt_a, yt_b):
                def go():
                    scp = t_scp[(qb * NJC + hp) % 2]
                    with nc.allow_low_precision(reason="bf16 softmax denom"):
                        nc.vector.tensor_copy(
                            scp[0:1, :], yt_a[hd:hd + 1, :])
                        nc.vector.tensor_copy(
                            scp[hd:hd + 1, :], yt_b[0:1, :])
                    bc = pmm.tile([P, 2 * TQB], F32, tag="sc", name="bc")
                    nc.tensor.matmul(bc[:, 0:TQB], lhsT=t_ea, rhs=scp[:])
                    rcp = prcp.tile([P, TQB], F32, tag="rcpb")
                    nc.vector.reciprocal_approx_fast(rcp[:], bc[:, 0:TQB])
                    with nc.allow_low_precision(reason="bf16 y"):
                        nc.vector.tensor_mul(
                            t_yn[hp][0:hd, qb * TQB:(qb + 1) * TQB],
                            yt_a[0:hd, :], rcp[0:hd, :])
                        nc.vector.tensor_mul(
                            t_yn[hp][hd:2 * hd, qb * TQB:(qb + 1) * TQB],
                            yt_b[hd:2 * hd, :], rcp[hd:2 * hd, :])
                return go

            def q_outproj(tt):
                def go():
                    zev = pzev.tile([P, C], BF16, tag="zev")
                    for co in range(C // TQB):
                        zps = pps.tile([P, TQB], F32, tag="pp")
                        for jc in range(NJC):
                            nc.tensor.matmul(
                                zps[:],
                                lhsT=t_yn[jc][:, tt * P:(tt + 1) * P],
                                rhs=t_wp[:, jc * C + co * TQB:
                                         jc * C + (co + 1) * TQB],
                                start=(jc == 0),
                                stop=(jc == NJC - 1),
                            )
                        with nc.allow_low_precision(reason="bf16 z"):
                            nc.any.tensor_copy(
                                zev[:, co * TQB:(co + 1) * TQB], zps[:])
                    nc.sync.dma_start(
                        z[tt * P:(tt + 1) * P, :], zev[:])
                return go

            def proj_quanta(tb):
                qs = [q_vproj(tt) for tt in range(4 * tb, 4 * tb + 4)]
                for jc in range(NJC):
                    qs.append(q_qkproj(t_qrot[jc], t_wq, t_bq, jc, tb))
                    qs.append(q_qkproj(t_krot[jc], t_wk, t_bk, jc, tb))
                return qs

            def attn_quanta(qb):
                qs = []
                n_kc = min(T // P, (qb + 1) * (TQB // P))
                for hp in range(NJC):
                    yt_a = pyt.tile([P, TQB], F32, tag="yta")
                    yt_b = pyt.tile([P, TQB], F32, tag="ytb")
                    for kc in range(n_kc):
                        qs.append(q_chunk(qb, hp, kc, n_kc, yt_a, yt_b))
                    qs.append(q_tail(qb, hp, yt_a, yt_b))
                return qs

            def merge(primary, fillers):
                if not primary:
                    for f in fillers:
                        f()
                    return
                ratio = len(fillers) / len(primary)
                acc = 0.0
                fi = 0
                for p in primary:
                    p()
                    acc += ratio
                    while acc >= 1.0 and fi < len(fillers):
                        fillers[fi]()
                        fi += 1
                        acc -= 1.0
                while fi < len(fillers):
                    fillers[fi]()
                    fi += 1

            # ---- schedule ----
            for q in proj_quanta(0):
                q()
            for qb in range(NQB):
                fillers = []
                if qb + 1 < NQB:
                    fillers += proj_quanta(qb + 1)
                if qb > 0:
                    fillers += [q_outproj(tt)
                                for tt in range(4 * (qb - 1), 4 * qb)]
                merge(attn_quanta(qb), fillers)
            for tt in range(4 * (NQB - 1), 4 * NQB):
                q_outproj(tt)()


_ROPE_PERM = np.concatenate([np.arange(0, 64, 2), np.arange(1, 64, 2)])


def _host_inputs(x_b, Wq, bq, Wk, bk, Wv, bv, Wp, heads, T, C, hd):
    """Build the per-core DRAM input dict (numpy)."""
    HD = len(heads) * hd
    rows = np.concatenate([h * hd + _ROPE_PERM for h in heads])
    rows_nop = np.concatenate([np.arange(h * hd, (h + 1) * hd) for h in heads])

    # xq[tb*128+p, k*512+c] = x[tb*512+c, k*128+p]
    xq = np.ascontiguousarray(
        x_b.reshape(4, 512, 8, P).transpose(0, 3, 2, 1).reshape(512, 4096)
    ).astype(NPBF16)

    def wswz(wt):  # [C, HD] -> [128, 8*HD] with [p, k*HD+c] = wt[k*128+p, c]
        return np.ascontiguousarray(
            wt.reshape(8, P, HD).transpose(1, 0, 2).reshape(P, 8 * HD))

    wqT = wswz(Wq[rows].T).astype(NPBF16)
    wkT = wswz(Wk[rows].T).astype(NPBF16)
    wvT = wswz(Wv[rows_nop].T).astype(NPBF16)
    wpTf = Wp[:, rows_nop].T  # [HD, C]
    wpT = np.ascontiguousarray(
        wpTf.reshape(2, P, C).transpose(1, 0, 2).reshape(P, 2 * C)
    ).astype(NPBF16)

    j = np.arange(hd // 2, dtype=np.float64)
    inv_freq = 1.0 / (10000.0 ** (2.0 * j / hd))
    t = np.arange(T, dtype=np.float64)
    ang = t[:, None] * inv_freq[None, :]          # [T, 32]
    cos = np.cos(ang)
    sin = np.sin(ang)
    r = np.arange(P)
    cosq = cos[:, r % (hd // 2)].T.astype(np.float32)
    sgn = np.where((r % hd) < hd // 2, -1.0, 1.0)
    sinsq = (sin[:, r % (hd // 2)] * sgn[None, :]).T.astype(np.float32)

    pair = np.where((r % hd) < hd // 2, r + hd // 2, r - hd // 2)
    jmat = np.zeros((P, P), np.float32)
    jmat[pair, r] = 1.0
    tri = (np.arange(P)[None, :] >= np.arange(P)[:, None]).astype(np.float32)
    ea = np.zeros((P, P), np.float32)
    ea[(r // hd) * hd, r] = 1.0

    bqTh = bq[rows].reshape(HD // P, P).T.astype(np.float32)
    bkTh = bk[rows].reshape(HD // P, P).T.astype(np.float32)
    bvb = np.tile(bv[rows_nop][None, :], (P, 1)).astype(np.float32)
    bias = np.ascontiguousarray(
        np.concatenate([bqTh, bkTh, bvb], axis=1)).astype(np.float32)
    cmat = np.ascontiguousarray(
        np.concatenate([jmat, tri, ea], axis=1)).astype(NPBF16)

    return {
        "xq": xq, "wqT": wqT, "wkT": wkT, "wvT": wvT, "wpT": wpT,
        "cosq": np.ascontiguousarray(cosq).astype(NPBF16),
        "sinsq": np.ascontiguousarray(sinsq).astype(NPBF16),
        "cmat": cmat,
        "bias": bias,
    }


def make_core_inputs(x, Wq, bq, Wk, bk, Wv, bv, Wp, T=2048, C=1024, hd=64,
                     heads_per_core=4):
    in_maps = []
    for c in range(N_CORES):
        b = c // 4
        g = c % 4
        heads = list(range(g * heads_per_core, (g + 1) * heads_per_core))
        in_maps.append(_host_inputs(
            np.asarray(x[b]), Wq, bq, Wk, bk, Wv, bv, Wp, heads, T, C, hd))
    return in_maps


def kernel(x, Wq, bq, Wk, bk, Wv, bv, Wp, bp):
    x = np.asarray(x, np.float32)
    Wq = np.asarray(Wq, np.float32)
    bq = np.asarray(bq, np.float32)
    Wk = np.asarray(Wk, np.float32)
    bk = np.asarray(bk, np.float32)
    Wv = np.asarray(Wv, np.float32)
    bv = np.asarray(bv, np.float32)
    Wp = np.asarray(Wp, np.float32)
    bp = np.asarray(bp, np.float32)
    B, T, C = x.shape

    _patch_act_tables()
    nc = bacc.Bacc("TRN2", target_bir_lowering=False, debug=False,
                   num_devices=N_CORES)
    build_attention_kernel(nc, T=T, C=C)
    nc.compile()

    in_maps = make_core_inputs(x, Wq, bq, Wk, bk, Wv, bv, Wp, T=T, C=C)
    res = run_bass_kernel_spmd(nc, in_maps, list(range(N_CORES)))

    out = np.zeros((B, T, C), np.float32)
    for c in range(N_CORES):
        out[c // 4] += res.results[c]["z"].astype(np.float32)
    out += bp[None, None, :]
    return out


if __name__ == "__main__":
    import reference

    inputs = reference.setup_inputs()
    expected = np.asarray(reference.reference(**inputs))
    actual = kernel(**{k: np.asarray(v) for k, v in inputs.items()})
    err = np.abs(actual - expected).max() / np.abs(expected).max()
    print("Relative error:", err)
